# revision 41
# baseline (speedup 1.0000x reference)
"""Trainium2 Bass kernel for nn_DPSpikingDecoder — streaming-tail variant.

Same math as v5a (one long PE contraction against a host-precomputed
scan+pool kernel), but rows are ordered time-major: sub-DMA w holds
exactly window w ([128, 6, F], 6 KB partition lines).  Because the scan
kernel is shift-invariant (K[w, t] = f(24w - t), with contributions
beyond the previous window < 2^-25), window w's dp row is final once
sub-DMAs w-1 and w are matmul'd.  Each group of 5 windows therefore
retires mid-stream: its PSUM rows are copied out, transposed, and its
10 MLP-layer-1 matmuls run in the shadow of the DMA stream, leaving
only ~1 window of contraction + the tiny layer-2/softmax/scale tail
after the last byte lands.  W1 streams in 8 per-group slices, each a
group ahead of its first use, so the in-order PE queue never blocks.

Sharding: data-parallel over batch B=8 -> one sample per NeuronCore.
"""

import numpy as np
from contextlib import ExitStack

import concourse.bass as bass
import concourse.bacc as bacc
import concourse.tile as tile
from concourse import mybir
from concourse.bass_utils import run_bass_kernel_spmd

F32 = mybir.dt.float32
F32R = mybir.dt.float32r

B, C, T, F = 8, 32, 960, 256
L_DP, N_DP = 24, 12
W = T // L_DP            # 40 windows
H = 20                   # hidden dim of the MLP

R = C * T                # 30720 contraction rows per sample
CH = 128                 # rows per matmul chunk (= 4 time steps x 32 ch)
SW = 6                   # chunks per sub-DMA = one 24-step window
NW = W                   # 40 sub-DMAs
G = 5                    # windows per PSUM group
NG = W // G              # 8 groups


def _host_K():
    """K[w, t] in float64: differential pooling of the decayed scan."""
    t = np.arange(T)
    d = t[:, None] - t[None, :]
    Lmat = np.where(d >= 0, 0.5 ** np.clip(d, 0, None), 0.0)
    M = np.zeros((W, T))
    for w in range(W):
        M[w, w * L_DP + L_DP - N_DP : w * L_DP + L_DP] = 1.0 / N_DP
        M[w, w * L_DP : w * L_DP + N_DP] -= 1.0 / N_DP
    return M @ Lmat  # [W, T]


def _host_kt():
    """Shift-invariant kernel images, one [128, (5*6 + 6)*5] block:
    mains  ktm[p, (r*6+k)*5 + c] = f(24(c-r) - tloc)/C,  tloc = 4k + p//32
    tails  ktt[p, (30+k)*5 + 0]  = f(24 - tloc)/C  (cols 1..4 zero)
    where f(gap) = K[w, 24w - gap] for any deep w (shift invariance)."""
    K = _host_K()
    f = np.zeros(512)  # f[gap + 256]
    for gap in range(-119, 121):
        f[gap + 256] = K[20, 480 - gap]
    p = np.arange(128)
    tloc = lambda k: 4 * k + p // 32          # [128]
    img = np.zeros((128, (G * SW + SW) * G), dtype=np.float64)
    for r in range(G):
        for k in range(SW):
            for c in range(G):
                img[:, (r * SW + k) * G + c] = f[24 * (c - r) - tloc(k) + 256] / C
    for k in range(SW):
        img[:, (G * SW + k) * G + 0] = f[24 - tloc(k) + 256] / C
    return np.ascontiguousarray(img.astype(np.float32))


def _host_cimg(W2, b2):
    """Packed small consts, one contiguous [128, 101] DMA image:
    cols 0:40 eye(40) on parts 0:40; 40:80 [W2; b2] on parts 0:21;
    col 80 b1 placeholder (zeros, real b1 patched in kernel());
    cols 81:101 the 4-col-group summing matrix."""
    img = np.zeros((128, 101), dtype=np.float32)
    img[0:W, 0:W] = np.eye(W, dtype=np.float32)
    img[0:H, 40:80] = W2.astype(np.float32)
    img[H, 40:80] = b2.astype(np.float32)
    for j in range(4):
        for i in range(H):
            img[32 * j + i, 81 + i] = 1.0
    return img


def _build_program():
    nc = bacc.Bacc(None)
    x = nc.declare_dram_parameter("x", [NW, CH, SW, F], F32R, isOutput=False)
    kt = nc.declare_dram_parameter("kt", [128, (G * SW + SW) * G], F32R, isOutput=False)
    w1r = nc.declare_dram_parameter("w1r", [128, 2 * W * H], F32, isOutput=False)
    cimg = nc.declare_dram_parameter("cimg", [128, 101], F32, isOutput=False)
    y = nc.declare_dram_parameter("y", [W, F], F32, isOutput=True)

    with tile.TileContext(nc) as tc, ExitStack() as ctx:
        consts = ctx.enter_context(tc.tile_pool(name="consts", bufs=1))
        xs = ctx.enter_context(tc.tile_pool(name="xs", bufs=8))
        work = ctx.enter_context(tc.tile_pool(name="work", bufs=1))
        g_psp = ctx.enter_context(tc.tile_pool(name="g_ps", bufs=2, space="PSUM"))
        dpt_psp = ctx.enter_context(tc.tile_pool(name="dpt_ps", bufs=1, space="PSUM"))
        hp_psp = ctx.enter_context(tc.tile_pool(name="hp_ps", bufs=1, space="PSUM"))
        sm_ps = ctx.enter_context(tc.tile_pool(name="sm_ps", bufs=1, space="PSUM"))

        kt_sb = consts.tile([128, G * SW + SW, G], F32R)
        nc.sync.dma_start(
            out=kt_sb, in_=kt[:].rearrange("p (q c) -> p q c", c=G)
        )
        ci_sb = consts.tile([128, 101], F32)
        nc.scalar.dma_start(out=ci_sb, in_=cimg[:])
        eye5 = ci_sb[0:G, 0:G]
        w2b_sb = ci_sb[0 : H + 1, 40:80]
        b1_sb = ci_sb[0:H, 80:81]
        sel_sb = ci_sb[:, 81:101]
        w1_sb = consts.tile([128, 2 * W * H], F32)

        h_aug = work.tile([H + 1, 1], F32)
        nc.vector.memset(h_aug, 1.0)
        dp_sb = work.tile([W, F], F32)
        dpT_sb = work.tile([128, 2, W], F32)
        hp_ps = hp_psp.tile([128, 1], F32)

        g_tiles = [None] * NG
        xt_prev = None
        # PE duty per window is ~65%, low enough that HAM throttles the
        # PE to its ~2x-slower cold rate for the whole stream; a few
        # harmless matmuls on resident kt data per window keep it dense
        # and warm, so the trailing windows and the MLP tail run fast.
        scr_ps = sm_ps.tile([G, G * SW * G], F32)

        for w in range(NW):
            g, r = divmod(w, G)
            xt = xs.tile([CH, SW, F], F32R)
            eng = nc.sync if w % 2 == 0 else nc.scalar
            eng.dma_start(out=xt, in_=x[w])
            # stream W1 in per-group slices, one group ahead of use
            if r == 0:
                for sg in ([0, 1] if g == 0 else [g + 1]):
                    if sg < NG:
                        oeng = nc.scalar if w % 2 == 0 else nc.sync
                        oeng.dma_start(
                            out=w1_sb[:, sg * 2 * G * H : (sg + 1) * 2 * G * H],
                            in_=w1r[:, sg * 2 * G * H : (sg + 1) * 2 * G * H],
                        )
            if r == 0:
                gt_new = g_psp.tile([G, F], F32)
                g_tiles[g] = gt_new
                if g > 0:
                    # decay tails of window 5g-1 into G_g row 0; the first
                    # is full-width with start=True so it initializes all
                    # 5 rows (cols 1..4 of ktt are zero)
                    for k in range(SW):
                        nc.tensor.matmul(
                            g_tiles[g],
                            lhsT=kt_sb[:, G * SW + k, :],
                            rhs=xt_prev[:, k, :],
                            start=(k == 0),
                            stop=False,
                        )
            for k in range(SW):
                nc.tensor.matmul(
                    g_tiles[g],
                    lhsT=kt_sb[:, r * SW + k, :],
                    rhs=xt[:, k, :],
                    start=(w == 0 and k == 0),
                    stop=(r == G - 1 and k == SW - 1),
                )
            xt_prev = xt
            for u in range(5):
                nc.tensor.matmul(
                    scr_ps,
                    lhsT=kt_sb[:, u, :],
                    rhs=kt_sb[:, 0 : G * SW, :],
                    start=True,
                    stop=True,
                )
            if r == G - 1:
                # group g is final: retire it in the stream's shadow.
                # DVE/PE ops need base partition 0, so stage the 5 rows at
                # partition 0 and assemble dp_sb via a tiny SBUF->SBUF DMA
                # (DMAs have no base-partition restriction).
                stage = work.tile([G, F], F32, tag="stage", bufs=2)
                nc.vector.tensor_copy(stage, g_tiles[g])
                nc.scalar.dma_start(out=dp_sb[G * g : G * (g + 1), :], in_=stage)
                dpT_ps = dpt_psp.tile([128, 2, G], F32)
                for e in range(2):
                    nc.tensor.transpose(
                        dpT_ps[:, e, :],
                        stage[:, e * 128 : (e + 1) * 128],
                        eye5,
                    )
                nc.vector.tensor_copy(dpT_sb[:, :, G * g : G * (g + 1)], dpT_ps)
                for m in range(2 * G * g, 2 * G * (g + 1)):
                    wi, e = divmod(m, 2)
                    j = m % 4
                    nc.tensor.matmul(
                        hp_ps[32 * j : 32 * j + H, :],
                        lhsT=w1_sb[:, m * H : (m + 1) * H],
                        rhs=dpT_sb[:, e, wi : wi + 1],
                        start=(m < 4),
                        stop=(m >= 2 * W - 4),
                        tile_position=(0, 32 * j),
                    )

        # ---- tiny MLP tail: only layer 2 + softmax + scale remain ----
        hp_sb = work.tile([128, 1], F32)
        nc.vector.tensor_copy(hp_sb, hp_ps)
        h_ps = sm_ps.tile([H, 1], F32)
        nc.tensor.matmul(h_ps, lhsT=sel_sb, rhs=hp_sb, start=True, stop=True)
        nc.scalar.activation(
            h_aug[0:H, :], h_ps, mybir.ActivationFunctionType.Relu, bias=b1_sb
        )
        a2_ps = sm_ps.tile([1, W], F32)
        nc.tensor.matmul(a2_ps, lhsT=h_aug, rhs=w2b_sb, start=True, stop=True)
        e_sb = work.tile([1, W], F32)
        ssum = work.tile([1, 1], F32)
        nc.scalar.activation(
            e_sb, a2_ps, mybir.ActivationFunctionType.Exp, accum_out=ssum[:]
        )
        rin = work.tile([1, 1], F32)
        nc.vector.reciprocal(rin, ssum)
        ta_sb = work.tile([1, W], F32)
        nc.vector.tensor_scalar_mul(ta_sb, e_sb, rin[:])
        taT_ps = sm_ps.tile([W, 1], F32)
        nc.tensor.transpose(taT_ps, ta_sb, ci_sb[0:1, 0:1])
        ta_col = work.tile([W, 1], F32)
        nc.vector.tensor_copy(ta_col, taT_ps)
        att = work.tile([W, F], F32)
        for e2 in range(2):
            nc.vector.tensor_scalar_mul(
                att[:, e2 * 128 : (e2 + 1) * 128],
                dp_sb[:, e2 * 128 : (e2 + 1) * 128],
                ta_col[:],
            )
        nc.sync.dma_start(out=y[:], in_=att[:])

    nc.compile()
    return nc


_CACHED = {}


def _get_program():
    if "nc" not in _CACHED:
        _CACHED["nc"] = _build_program()
        _CACHED["kt"] = _host_kt()
    return _CACHED["nc"]


def _in_maps(spikes, W1, b1, W2, b2):
    spikes = np.asarray(spikes, dtype=np.float32)
    W1 = np.asarray(W1, dtype=np.float32)
    b1 = np.asarray(b1, dtype=np.float32)
    W2 = np.asarray(W2, dtype=np.float32)
    b2 = np.asarray(b2, dtype=np.float32)
    _get_program()
    w1r = np.ascontiguousarray(
        W1.reshape(W, 2, 128, H).transpose(2, 0, 1, 3).reshape(128, 2 * W * H)
    )
    cimg = _host_cimg(W2, b2)
    cimg[0:H, 80] = b1
    shared = {"kt": _CACHED["kt"], "w1r": w1r, "cimg": cimg}
    # time-major partition layout:
    #   x[w, 32*pp + c, k, f] = spikes[b, c, 24w + 4k + pp, f]
    maps = []
    for b in range(B):
        tcf = spikes[b].transpose(1, 0, 2)                 # [T, C, F]
        x_ = np.ascontiguousarray(
            tcf.reshape(NW, SW, 4, C, F).transpose(0, 2, 3, 1, 4).reshape(NW, CH, SW, F)
        )
        maps.append({"x": x_, **shared})
    return maps


def kernel(spikes, W1, b1, W2, b2):
    in_maps = _in_maps(spikes, W1, b1, W2, b2)
    res = run_bass_kernel_spmd(_get_program(), in_maps, list(range(B)))
    out = np.stack([np.asarray(res.results[i]["y"]).reshape(W * F) for i in range(B)])
    return out.astype(np.float32)


# revision 46
# speedup vs baseline: 1.0516x; 1.0516x over previous
"""Trainium2 Bass kernel for nn_DPSpikingDecoder — streaming-tail variant.

Same math as v5a (one long PE contraction against a host-precomputed
scan+pool kernel), but rows are ordered time-major: sub-DMA w holds
exactly window w ([128, 6, F], 6 KB partition lines).  Because the scan
kernel is shift-invariant (K[w, t] = f(24w - t), with contributions
beyond the previous window < 2^-25), window w's dp row is final once
sub-DMAs w-1 and w are matmul'd.  Each group of 5 windows therefore
retires mid-stream: its PSUM rows are copied out, transposed, and its
10 MLP-layer-1 matmuls run in the shadow of the DMA stream, leaving
only ~1 window of contraction + the tiny layer-2/softmax/scale tail
after the last byte lands.  W1 streams in 8 per-group slices, each a
group ahead of its first use, so the in-order PE queue never blocks.

Sharding: data-parallel over batch B=8 -> one sample per NeuronCore.
"""

import numpy as np
from contextlib import ExitStack

import concourse.bass as bass
import concourse.bacc as bacc
import concourse.tile as tile
from concourse import mybir
from concourse.bass_utils import run_bass_kernel_spmd

F32 = mybir.dt.float32
F32R = mybir.dt.float32r

B, C, T, F = 8, 32, 960, 256
L_DP, N_DP = 24, 12
W = T // L_DP            # 40 windows
H = 20                   # hidden dim of the MLP

R = C * T                # 30720 contraction rows per sample
CH = 128                 # rows per matmul chunk (= 4 time steps x 32 ch)
SW = 6                   # chunks per sub-DMA = one 24-step window
NW = W                   # 40 sub-DMAs
G = 5                    # windows per PSUM group
NG = W // G              # 8 groups


def _host_K():
    """K[w, t] in float64: differential pooling of the decayed scan."""
    t = np.arange(T)
    d = t[:, None] - t[None, :]
    Lmat = np.where(d >= 0, 0.5 ** np.clip(d, 0, None), 0.0)
    M = np.zeros((W, T))
    for w in range(W):
        M[w, w * L_DP + L_DP - N_DP : w * L_DP + L_DP] = 1.0 / N_DP
        M[w, w * L_DP : w * L_DP + N_DP] -= 1.0 / N_DP
    return M @ Lmat  # [W, T]


def _host_kt():
    """Shift-invariant kernel images, one [128, (5*6 + 6)*5] block:
    mains  ktm[p, (r*6+k)*5 + c] = f(24(c-r) - tloc)/C,  tloc = 4k + p//32
    tails  ktt[p, (30+k)*5 + 0]  = f(24 - tloc)/C  (cols 1..4 zero)
    where f(gap) = K[w, 24w - gap] for any deep w (shift invariance)."""
    K = _host_K()
    f = np.zeros(512)  # f[gap + 256]
    for gap in range(-119, 121):
        f[gap + 256] = K[20, 480 - gap]
    p = np.arange(128)
    tloc = lambda k: 4 * k + p // 32          # [128]
    img = np.zeros((128, (G * SW + SW) * G), dtype=np.float64)
    for r in range(G):
        for k in range(SW):
            for c in range(G):
                img[:, (r * SW + k) * G + c] = f[24 * (c - r) - tloc(k) + 256] / C
    for k in range(SW):
        img[:, (G * SW + k) * G + 0] = f[24 - tloc(k) + 256] / C
    return np.ascontiguousarray(img.astype(np.float32))


def _host_cimg(W2, b2):
    """Packed small consts, one contiguous [128, 101] DMA image:
    cols 0:40 eye(40) on parts 0:40; 40:80 [W2; b2] on parts 0:21;
    col 80 b1 placeholder (zeros, real b1 patched in kernel());
    cols 81:101 the 4-col-group summing matrix."""
    img = np.zeros((128, 101), dtype=np.float32)
    img[0:W, 0:W] = np.eye(W, dtype=np.float32)
    img[0:H, 40:80] = W2.astype(np.float32)
    img[H, 40:80] = b2.astype(np.float32)
    for j in range(4):
        for i in range(H):
            img[32 * j + i, 81 + i] = 1.0
    return img


def _build_program():
    nc = bacc.Bacc(None)
    x = nc.declare_dram_parameter("x", [NW, CH, SW, F], F32R, isOutput=False)
    kt = nc.declare_dram_parameter("kt", [128, (G * SW + SW) * G], F32R, isOutput=False)
    w1r = nc.declare_dram_parameter("w1r", [128, 2 * W * H], F32, isOutput=False)
    cimg = nc.declare_dram_parameter("cimg", [128, 101], F32, isOutput=False)
    y = nc.declare_dram_parameter("y", [W, F], F32, isOutput=True)

    with tile.TileContext(nc) as tc, ExitStack() as ctx:
        consts = ctx.enter_context(tc.tile_pool(name="consts", bufs=1))
        xs = ctx.enter_context(tc.tile_pool(name="xs", bufs=12))
        work = ctx.enter_context(tc.tile_pool(name="work", bufs=1))
        g_psp = ctx.enter_context(tc.tile_pool(name="g_ps", bufs=2, space="PSUM"))
        dpt_psp = ctx.enter_context(tc.tile_pool(name="dpt_ps", bufs=2, space="PSUM"))
        hp_psp = ctx.enter_context(tc.tile_pool(name="hp_ps", bufs=1, space="PSUM"))
        sm_ps = ctx.enter_context(tc.tile_pool(name="sm_ps", bufs=1, space="PSUM"))

        # kt/cimg are tiny; their DMAs are emitted after sub-DMA 0's so
        # the bulk x descriptors lead the HWDGE generation queue and the
        # SDMA ramp starts ~3 us earlier (the PE lags the stream anyway).
        kt_sb = consts.tile([128, G * SW + SW, G], F32R)
        ci_sb = consts.tile([128, 101], F32)
        eye5 = ci_sb[0:G, 0:G]
        w2b_sb = ci_sb[0 : H + 1, 40:80]
        b1_sb = ci_sb[0:H, 80:81]
        sel_sb = ci_sb[:, 81:101]
        w1_sb = consts.tile([128, 2 * W * H], F32)

        h_aug = work.tile([H + 1, 1], F32)
        nc.vector.memset(h_aug, 1.0)
        dp_sb = work.tile([W, F], F32)
        dpT_sb = work.tile([128, 2, W], F32)
        hp_ps = hp_psp.tile([128, 1], F32)

        g_tiles = [None] * NG
        xt_prev = None

        for w in range(NW):
            g, r = divmod(w, G)
            xt = xs.tile([CH, SW, F], F32R)
            eng = nc.sync if w % 2 == 0 else nc.scalar
            eng.dma_start(out=xt, in_=x[w])
            if w == 0:
                nc.sync.dma_start(
                    out=kt_sb, in_=kt[:].rearrange("p (q c) -> p q c", c=G)
                )
                nc.scalar.dma_start(out=ci_sb, in_=cimg[:])
            # stream W1 in per-group slices, one group ahead of use
            if r == 0:
                for sg in ([0, 1] if g == 0 else [g + 1]):
                    if sg < NG:
                        oeng = nc.scalar if w % 2 == 0 else nc.sync
                        oeng.dma_start(
                            out=w1_sb[:, sg * 2 * G * H : (sg + 1) * 2 * G * H],
                            in_=w1r[:, sg * 2 * G * H : (sg + 1) * 2 * G * H],
                        )
            if r == 0:
                gt_new = g_psp.tile([G, F], F32)
                g_tiles[g] = gt_new
                if g > 0:
                    # decay tails of window 5g-1 into G_g row 0; the first
                    # is full-width with start=True so it initializes all
                    # 5 rows (cols 1..4 of ktt are zero)
                    for k in range(SW):
                        nc.tensor.matmul(
                            g_tiles[g],
                            lhsT=kt_sb[:, G * SW + k, :],
                            rhs=xt_prev[:, k, :],
                            start=(k == 0),
                            stop=False,
                        )
            for k in range(SW):
                nc.tensor.matmul(
                    g_tiles[g],
                    lhsT=kt_sb[:, r * SW + k, :],
                    rhs=xt[:, k, :],
                    start=(w == 0 and k == 0),
                    stop=(r == G - 1 and k == SW - 1),
                )
            xt_prev = xt
            if r == G - 1:
                # group g is final: retire it in the stream's shadow.
                # DVE/PE ops need base partition 0, so stage the 5 rows at
                # partition 0 and assemble dp_sb via a tiny SBUF->SBUF DMA
                # (DMAs have no base-partition restriction).
                stage = work.tile([G, F], F32, tag="stage", bufs=2)
                nc.vector.tensor_copy(stage, g_tiles[g])
                nc.scalar.dma_start(out=dp_sb[G * g : G * (g + 1), :], in_=stage)
                dpT_ps = dpt_psp.tile([128, 2, G], F32)
                for e in range(2):
                    nc.tensor.transpose(
                        dpT_ps[:, e, :],
                        stage[:, e * 128 : (e + 1) * 128],
                        eye5,
                    )
                nc.vector.tensor_copy(dpT_sb[:, :, G * g : G * (g + 1)], dpT_ps)
                for m in range(2 * G * g, 2 * G * (g + 1)):
                    wi, e = divmod(m, 2)
                    j = m % 4
                    nc.tensor.matmul(
                        hp_ps[32 * j : 32 * j + H, :],
                        lhsT=w1_sb[:, m * H : (m + 1) * H],
                        rhs=dpT_sb[:, e, wi : wi + 1],
                        start=(m < 4),
                        stop=(m >= 2 * W - 4),
                        tile_position=(0, 32 * j),
                    )

        # ---- tiny MLP tail: only layer 2 + softmax + scale remain ----
        hp_sb = work.tile([128, 1], F32)
        nc.vector.tensor_copy(hp_sb, hp_ps)
        h_ps = sm_ps.tile([H, 1], F32)
        nc.tensor.matmul(h_ps, lhsT=sel_sb, rhs=hp_sb, start=True, stop=True)
        nc.scalar.activation(
            h_aug[0:H, :], h_ps, mybir.ActivationFunctionType.Relu, bias=b1_sb
        )
        a2_ps = sm_ps.tile([1, W], F32)
        nc.tensor.matmul(a2_ps, lhsT=h_aug, rhs=w2b_sb, start=True, stop=True)
        e_sb = work.tile([1, W], F32)
        ssum = work.tile([1, 1], F32)
        nc.scalar.activation(
            e_sb, a2_ps, mybir.ActivationFunctionType.Exp, accum_out=ssum[:]
        )
        rin = work.tile([1, 1], F32)
        nc.vector.reciprocal(rin, ssum)
        ta_sb = work.tile([1, W], F32)
        nc.vector.tensor_scalar_mul(ta_sb, e_sb, rin[:])
        taT_ps = sm_ps.tile([W, 1], F32)
        nc.tensor.transpose(taT_ps, ta_sb, ci_sb[0:1, 0:1])
        ta_col = work.tile([W, 1], F32)
        nc.vector.tensor_copy(ta_col, taT_ps)
        att = work.tile([W, F], F32)
        for e2 in range(2):
            nc.vector.tensor_scalar_mul(
                att[:, e2 * 128 : (e2 + 1) * 128],
                dp_sb[:, e2 * 128 : (e2 + 1) * 128],
                ta_col[:],
            )
            eng2 = nc.sync if e2 == 0 else nc.scalar
            eng2.dma_start(
                out=y[:, e2 * 128 : (e2 + 1) * 128],
                in_=att[:, e2 * 128 : (e2 + 1) * 128],
            )

    nc.compile()
    return nc


_CACHED = {}


def _get_program():
    if "nc" not in _CACHED:
        _CACHED["nc"] = _build_program()
        _CACHED["kt"] = _host_kt()
    return _CACHED["nc"]


def _in_maps(spikes, W1, b1, W2, b2):
    spikes = np.asarray(spikes, dtype=np.float32)
    W1 = np.asarray(W1, dtype=np.float32)
    b1 = np.asarray(b1, dtype=np.float32)
    W2 = np.asarray(W2, dtype=np.float32)
    b2 = np.asarray(b2, dtype=np.float32)
    _get_program()
    w1r = np.ascontiguousarray(
        W1.reshape(W, 2, 128, H).transpose(2, 0, 1, 3).reshape(128, 2 * W * H)
    )
    cimg = _host_cimg(W2, b2)
    cimg[0:H, 80] = b1
    shared = {"kt": _CACHED["kt"], "w1r": w1r, "cimg": cimg}
    # time-major partition layout:
    #   x[w, 32*pp + c, k, f] = spikes[b, c, 24w + 4k + pp, f]
    maps = []
    for b in range(B):
        tcf = spikes[b].transpose(1, 0, 2)                 # [T, C, F]
        x_ = np.ascontiguousarray(
            tcf.reshape(NW, SW, 4, C, F).transpose(0, 2, 3, 1, 4).reshape(NW, CH, SW, F)
        )
        maps.append({"x": x_, **shared})
    return maps


def kernel(spikes, W1, b1, W2, b2):
    in_maps = _in_maps(spikes, W1, b1, W2, b2)
    res = run_bass_kernel_spmd(_get_program(), in_maps, list(range(B)))
    out = np.stack([np.asarray(res.results[i]["y"]).reshape(W * F) for i in range(B)])
    return out.astype(np.float32)
